# revision 1
# baseline (speedup 1.0000x reference)
"""Trainium2 Bass kernel for DynamicGCN (4x NNConv + edge head), 8-core SPMD.

Strategy: edges are sharded by destination-node range (col) across 8 cores, so
the scatter-mean aggregation is core-local. Per layer, each core:
  node matmuls (own 1250-node shard, feature-major) -> PE-transpose to a
  node-major gather table -> AllGather across cores -> dma_gather per edge ->
  edge-NN (h, W) on PE -> per-edge bmm contraction on DVE (0-step broadcast
  multiply + grouped reduce) -> scatter-mean as PE matmuls into a resident
  PSUM accumulator using host-built inverse-degree one-hot blocks.
The final edge head uses bf16 transposed-gathers (feature-major) and two
accumulating matmuls.
"""

import os
import sys
import hashlib

import numpy as np

sys.path.insert(0, "/opt/trn_rl_repo")

import concourse.bass as bass
import concourse.bacc as bacc
import concourse.mybir as mybir
import concourse.tile as tile
from concourse.library_config import mlp as gpsimd_mlp_lib
from concourse.bass_utils import run_bass_kernel_spmd
from contextlib import ExitStack

# Problem dims (hardcoded per contract)
N = 10000
E = 160000
IN_C = 16
IN_E = 8
DIMS = [32, 16, 16, 16]
IND = [16, 32, 16, 16]
ENN_H = 128
EMLP_H = 16

NCOR = 8
PROD_BF16 = bool(int(os.environ.get("PROD_BF16", "1")))
NS = N // NCOR          # 1250 nodes per shard
NSP = 1280              # padded shard rows (10 chunks of 128)
NG = NCOR * NSP         # 10240 gather-table rows
WIN = 48                # scatter column window
GW = 64                 # gather payload floats (256B)
GWB = 128               # bf16 head payload elems (256B)

f32 = mybir.dt.float32
bf16 = mybir.dt.bfloat16
i16 = mybir.dt.int16


def _wrap_idx(idx):
    """int16 gather-index layout: [16, n/16] wrap, tiled to 128 partitions."""
    assert idx.size % 16 == 0
    w = idx.reshape(16, -1, order="F").astype(np.int16)
    return np.tile(w, (8, 1))


def _host_prep(x, edge_attr, edge_index, params, m_w0, m_b0, m_w1, m_b1):
    row = np.asarray(edge_index[0], dtype=np.int64)
    col = np.asarray(edge_index[1], dtype=np.int64)
    ea = np.asarray(edge_attr, dtype=np.float32)
    xf = np.asarray(x, dtype=np.float32)

    deg = np.bincount(col, minlength=N).astype(np.float32)
    inv_deg = 1.0 / np.maximum(deg, 1.0)

    # --- shard + sort + tile with static window schedule ---
    shard = col // NS
    per_core = []
    for c in range(NCOR):
        eids = np.nonzero(shard == c)[0]
        eids = eids[np.argsort(col[eids], kind="stable")]
        per_core.append(eids)

    def base_of(t):
        return max(0, min(8 * t - 20, NSP - WIN))

    # greedy fill under the static base schedule
    tiles_per_core = []
    for c in range(NCOR):
        eids = per_core[c]
        lc = col[eids] - NS * c
        tiles = []
        p = 0
        t = 0
        while p < len(eids):
            b = base_of(t)
            hi = b + WIN
            q = p
            while q < len(eids) and q - p < 128 and lc[q] < hi:
                q += 1
            assert q == p or lc[p] >= b, (
                f"window schedule violated: core {c} tile {t} col {lc[p]} < base {b}"
            )
            tiles.append(eids[p:q])
            p = q
            t += 1
        tiles_per_core.append(tiles)

    T = max(len(tl) for tl in tiles_per_core)
    T = 4 * ((T + 3) // 4)  # chunkable by 4
    EP = T * 128
    bases = [base_of(t) for t in range(T)]

    # --- per-core device arrays ---
    import ml_dtypes as _mld
    ea_sh = np.zeros((NCOR, 8, EP), dtype=_mld.bfloat16)
    hotv = np.zeros((NCOR, 128, T, WIN), dtype=np.float32)
    ridx = np.zeros((NCOR, 128, EP // 16), dtype=np.int16)
    cidx = np.zeros((NCOR, 128, EP // 16), dtype=np.int16)
    xT0 = np.zeros((NCOR, IND[0] + 1, NSP), dtype=np.float32)
    keep = []  # (positions-in-original-E, padded positions) per core
    for c in range(NCOR):
        gr = np.zeros(EP, dtype=np.int64)
        gc = np.zeros(EP, dtype=np.int64)
        orig = np.full(EP, -1, dtype=np.int64)
        for t, te in enumerate(tiles_per_core[c]):
            s = t * 128
            k = len(te)
            if k:
                ea_sh[c, :, s : s + k] = ea[te].T
                gr[s : s + k] = (row[te] // NS) * NSP + row[te] % NS
                gc[s : s + k] = (col[te] // NS) * NSP + col[te] % NS
                lcol = col[te] - NS * c
                hotv[c, np.arange(k), t, lcol - bases[t]] = inv_deg[col[te]]
                orig[s : s + k] = te
        ridx[c] = _wrap_idx(gr)
        cidx[c] = _wrap_idx(gc)
        xT0[c, :IND[0], :NS] = xf[NS * c : NS * (c + 1)].T
        xT0[c, IND[0], :NS] = 1.0
        keep.append(orig)

    import ml_dtypes
    # --- folded weights (shared across cores) ---
    Wnode, Wroot, ew1T, eb1, ew2pT = [], [], [], [], []
    for L in range(4):
        pw, pb, ew1, eb1_, ew2, eb2, rw, cb = params[L]
        d_in, d = IND[L], DIMS[L]
        B = eb2.reshape(d, d)
        xb_off = 64 if d == 32 else 32
        wn = np.zeros((d_in + 1, xb_off + d), dtype=np.float32)
        wn[:d_in, :d] = pw.T
        wn[d_in, :d] = pb
        wn[d_in, d] = 1.0
        wn[:d_in, xb_off:] = pw.T @ B
        wn[d_in, xb_off:] = pb @ B
        Wnode.append(wn)
        wr = np.zeros((d + 1, d), dtype=np.float32)
        wr[:d, :] = rw.T
        wr[d, :] = cb
        Wroot.append(wr)
        ew1T.append(np.ascontiguousarray(ew1.T).astype(ml_dtypes.bfloat16))
        eb1.append(eb1_.reshape(128, 1).astype(np.float32))
        # rhs[k, o*d+i] = ew2[i*d+o, k]
        ew2pT.append(
            np.ascontiguousarray(
                ew2.reshape(d, d, ENN_H).transpose(1, 0, 2).reshape(d * d, ENN_H).T
            ).astype(ml_dtypes.bfloat16)
        )
    wh0a = np.zeros((17, 16), dtype=np.float32)
    wh0a[:16] = m_w0.T
    wh0a[16] = m_b0
    wh0b = np.zeros((17, 16), dtype=np.float32)
    wh0b[:16] = m_w0.T
    wh1 = np.zeros((17, 1), dtype=np.float32)
    wh1[:16, 0] = m_w1[0]
    wh1[16, 0] = m_b1[0]
    id3 = np.tile(np.eye(32, dtype=np.float32), (3, 1))
    import ml_dtypes

    shared = {
        "wh0a": wh0a.astype(ml_dtypes.bfloat16),
        "wh0b": wh0b.astype(ml_dtypes.bfloat16),
        "wh1": wh1,
        "id3": id3,
        "ident16b": np.eye(16, dtype=np.float32).astype(ml_dtypes.bfloat16),
    }
    for L in range(4):
        shared[f"wnode{L}"] = Wnode[L]
        shared[f"wroot{L}"] = Wroot[L]
        shared[f"ew1T{L}"] = ew1T[L]
        shared[f"eb1{L}"] = eb1[L]
        shared[f"ew2pT{L}"] = ew2pT[L]

    in_maps = []
    for c in range(NCOR):
        m = {
            "ea_sh": ea_sh[c],
            "hotv": hotv[c],
            "ridx": ridx[c],
            "cidx": cidx[c],
            "xT0": xT0[c],
        }
        m.update(shared)
        in_maps.append(m)
    return T, bases, in_maps, keep


def _build_program(T, bases, repeat=1, skip_cc=False):
    EP = T * 128
    TC = T // 4  # tiles per gather chunk

    nc = bacc.Bacc(num_devices=NCOR)

    # I/O
    ea_d = nc.dram_tensor("ea_sh", [8, EP], bf16, kind="ExternalInput")
    hotv_d = nc.dram_tensor("hotv", [128, T, WIN], f32, kind="ExternalInput")
    ridx_d = nc.dram_tensor("ridx", [128, EP // 16], i16, kind="ExternalInput")
    cidx_d = nc.dram_tensor("cidx", [128, EP // 16], i16, kind="ExternalInput")
    xT0_d = nc.dram_tensor("xT0", [IND[0] + 1, NSP], f32, kind="ExternalInput")
    wh0a_d = nc.dram_tensor("wh0a", [17, 16], bf16, kind="ExternalInput")
    wh0b_d = nc.dram_tensor("wh0b", [17, 16], bf16, kind="ExternalInput")
    wh1_d = nc.dram_tensor("wh1", [17, 1], f32, kind="ExternalInput")
    id3_d = nc.dram_tensor("id3", [96, 32], f32, kind="ExternalInput")
    id16b_d = nc.dram_tensor("ident16b", [16, 16], bf16, kind="ExternalInput")
    wnode_d = [
        nc.dram_tensor(f"wnode{L}", [IND[L] + 1, (64 if DIMS[L] == 32 else 32) + DIMS[L]], f32, kind="ExternalInput")
        for L in range(4)
    ]
    wroot_d = [
        nc.dram_tensor(f"wroot{L}", [DIMS[L] + 1, DIMS[L]], f32, kind="ExternalInput")
        for L in range(4)
    ]
    ew1T_d = [nc.dram_tensor(f"ew1T{L}", [8, 128], bf16, kind="ExternalInput") for L in range(4)]
    eb1_d = [nc.dram_tensor(f"eb1{L}", [128, 1], f32, kind="ExternalInput") for L in range(4)]
    ew2pT_d = [
        nc.dram_tensor(f"ew2pT{L}", [128, DIMS[L] * DIMS[L]], bf16, kind="ExternalInput")
        for L in range(4)
    ]
    out_d = nc.dram_tensor("out", [1, EP], f32, kind="ExternalOutput")

    # internal DRAM
    xcat_in = [nc.dram_tensor(f"xcat_in{L}", [NSP, GW], f32) for L in range(4)]
    xag = [
        nc.dram_tensor(f"xag{L}", [NG, GW], f32, addr_space="Shared") for L in range(4)
    ]
    x4in = nc.dram_tensor("x4in", [NSP, GWB], bf16)
    x4ag = nc.dram_tensor("x4ag", [NG, GWB], bf16, addr_space="Shared")

    NCHUNK = [(0, 512), (512, 512), (1024, 256)]

    with tile.TileContext(nc) as tc, ExitStack() as ctx:
        cp = ctx.enter_context(tc.tile_pool(name="const", bufs=1))
        pg = ctx.enter_context(tc.tile_pool(name="gath", bufs=4))
        pw_ = ctx.enter_context(tc.tile_pool(name="work", bufs=2))
        ph4 = ctx.enter_context(tc.tile_pool(name="hpipe", bufs=6))
        px = ctx.enter_context(tc.tile_pool(name="xstate", bufs=2))
        ps_agg = ctx.enter_context(tc.tile_pool(name="psagg", bufs=1, space="PSUM"))
        ps_w = ctx.enter_context(tc.tile_pool(name="psw", bufs=2, space="PSUM"))
        ps_h = ctx.enter_context(tc.tile_pool(name="psh", bufs=1, space="PSUM"))

        nc.gpsimd.load_library(gpsimd_mlp_lib)

        # ---- resident constants ----
        hotv_sb = cp.tile([128, T, WIN], f32)
        nc.sync.dma_start(out=hotv_sb[:], in_=hotv_d[:])
        ridx_sb = cp.tile([128, EP // 16], i16)
        nc.sync.dma_start(out=ridx_sb[:], in_=ridx_d[:])
        cidx_sb = cp.tile([128, EP // 16], i16)
        nc.sync.dma_start(out=cidx_sb[:], in_=cidx_d[:])
        wh0a_sb = cp.tile([17, 16], bf16)
        nc.sync.dma_start(out=wh0a_sb[:], in_=wh0a_d[:])
        wh0b_sb = cp.tile([17, 16], bf16)
        nc.sync.dma_start(out=wh0b_sb[:], in_=wh0b_d[:])
        wh1_sb = cp.tile([17, 1], f32)
        nc.sync.dma_start(out=wh1_sb[:], in_=wh1_d[:])
        id3_sb = cp.tile([96, 32], f32)
        nc.sync.dma_start(out=id3_sb[:], in_=id3_d[:])
        id16b_sb = cp.tile([16, 16], bf16)
        nc.sync.dma_start(out=id16b_sb[:], in_=id16b_d[:])
        wnode_sb, wroot_sb, ew1T_sb, eb1_sb, ew2pT_sb = [], [], [], [], []
        for L in range(4):
            t_ = cp.tile([IND[L] + 1, (64 if DIMS[L] == 32 else 32) + DIMS[L]], f32, tag=f"wnode{L}")
            nc.sync.dma_start(out=t_[:], in_=wnode_d[L][:])
            wnode_sb.append(t_)
            t_ = cp.tile([DIMS[L] + 1, DIMS[L]], f32, tag=f"wroot{L}")
            nc.sync.dma_start(out=t_[:], in_=wroot_d[L][:])
            wroot_sb.append(t_)
            t_ = cp.tile([8, 128], bf16, tag=f"ew1T{L}")
            nc.sync.dma_start(out=t_[:], in_=ew1T_d[L][:])
            ew1T_sb.append(t_)
            t_ = cp.tile([128, 1], f32, tag=f"eb1{L}")
            nc.sync.dma_start(out=t_[:], in_=eb1_d[L][:])
            eb1_sb.append(t_)
            t_ = cp.tile([128, DIMS[L] * DIMS[L]], bf16, tag=f"ew2pT{L}")
            nc.sync.dma_start(out=t_[:], in_=ew2pT_d[L][:])
            ew2pT_sb.append(t_)
        zz = cp.tile([128, 512], f32)
        nc.vector.memset(zz[:], 0.0)
        xcat_sb = cp.tile([128, 10, GW], f32)
        nc.vector.memset(xcat_sb[:], 0.0)
        x4cat_sb = cp.tile([128, 10, GWB], bf16)
        nc.vector.memset(x4cat_sb[:], 0.0)
        nc.vector.memset(x4cat_sb[:, :, 16:17], 1.0)

        for _rep in range(repeat):
            # layer-0 x state
            xT_cur = px.tile([IND[0] + 1, NSP], f32, tag="xT0in")
            nc.sync.dma_start(out=xT_cur[:], in_=xT0_d[:])

            for L in range(4):
                d_in, d = IND[L], DIMS[L]
                A = d_in + 1
                XB = 64 if d == 32 else 32
                M2 = XB + d
                D2 = d * d

                # ---- node matmuls: xpxb = [xprojT; ones; xbT] ----
                xpxb_sb = pw_.tile([M2, NSP], f32, tag="xpxb")
                for (o, n) in NCHUNK:
                    mm = ps_w.tile([M2, n], f32, tag="w", space="PSUM")
                    nc.tensor.matmul(
                        mm[:], wnode_sb[L][:], xT_cur[:A, o : o + n], start=True, stop=True
                    )
                    nc.scalar.activation(
                        xpxb_sb[:, o : o + n], mm[:], mybir.ActivationFunctionType.Copy
                    )

                # ---- transpose shard to node-major gather payload ----
                for ch in range(10):
                    sl = slice(ch * 128, (ch + 1) * 128)
                    tp = ps_h.tile([128, 512], f32, tag="h", space="PSUM")
                    nc.tensor.transpose(
                        out=tp[:, 0:d], in_=xpxb_sb[0:d, sl], identity=id3_sb[0:d, 0:d]
                    )
                    nc.scalar.activation(
                        xcat_sb[:, ch, 0:d], tp[:, 0:d], mybir.ActivationFunctionType.Copy
                    )
                    tp2 = ps_h.tile([128, 512], f32, tag="h", space="PSUM")
                    nc.tensor.transpose(
                        out=tp2[:, 0:d],
                        in_=xpxb_sb[XB:M2, sl],
                        identity=id3_sb[XB : XB + d, 0:d],
                    )
                    nc.scalar.activation(
                        xcat_sb[:, ch, d : 2 * d],
                        tp2[:, 0:d],
                        mybir.ActivationFunctionType.Copy,
                    )
                nc.sync.dma_start(
                    out=xcat_in[L].ap().rearrange("(c p) w -> p c w", p=128),
                    in_=xcat_sb[:],
                )
                nc.gpsimd.collective_compute(
                    "AllGather",
                    mybir.AluOpType.bypass,
                    replica_groups=[list(range(NCOR))],
                    ins=[xcat_in[L][:]],
                    outs=[xag[L][:]],
                )

                # ---- zero-init aggT ----
                aggT = ps_agg.tile([d, NSP], f32, tag="agg", space="PSUM")
                for (o, n) in NCHUNK:
                    nc.tensor.matmul(
                        aggT[:, o : o + n], zz[:, 0:d], zz[:, 0:n], start=True, stop=False
                    )

                # ---- edge phase ----
                for ck in range(4):
                    gbuf = pg.tile([128, TC, GW], f32, tag="g")
                    nc.gpsimd.dma_gather(
                        gbuf[:],
                        xag[L][:],
                        ridx_sb[:, ck * TC * 8 : (ck + 1) * TC * 8],
                        TC * 128,
                        TC * 128,
                        GW,
                        single_packet=False,
                    )
                    for t4 in range(TC // 4):
                        g0 = ck * TC + t4 * 4  # global tile of first in group
                        es = g0 * 128  # edge offset
                        ea_g = ph4.tile([8, 512], bf16, tag="ea")
                        nc.sync.dma_start(out=ea_g[:], in_=ea_d[:, es : es + 512])
                        hp = ps_h.tile([128, 512], f32, tag="h", space="PSUM")
                        nc.tensor.matmul(
                            hp[:], ew1T_sb[L][:], ea_g[:], start=True, stop=True
                        )
                        hs = ph4.tile([128, 512], bf16, tag="hs")
                        nc.scalar.activation(
                            hs[:], hp[:], mybir.ActivationFunctionType.Relu, bias=eb1_sb[L][:]
                        )
                        if d == 16:
                            # batched: 4 tiles share one [128,1024] W tile and
                            # one DVE mult+reduce (amortizes per-op overhead)
                            tl0 = t4 * 4
                            wp = ps_w.tile([128, 1024], f32, tag="w", space="PSUM")
                            for tt in range(4):
                                nc.tensor.matmul(
                                    wp[:, tt * 256 : (tt + 1) * 256],
                                    hs[:, tt * 128 : (tt + 1) * 128],
                                    ew2pT_sb[L][:],
                                    start=True,
                                    stop=True,
                                )
                            prod = pw_.tile([128, 1024], bf16 if PROD_BF16 else f32, tag="prod")
                            xp0 = gbuf[:, tl0, 0:d]
                            xp_b = bass.AP(
                                tensor=xp0.tensor,
                                offset=xp0.offset,
                                ap=[xp0.ap[0], [GW, 4], [0, d], [1, d]],
                            )
                            nc.vector.tensor_tensor(
                                out=prod[:], in0=wp[:], in1=xp_b, op=mybir.AluOpType.mult
                            )
                            msgt4 = pw_.tile([128, 64], f32, tag="msg")
                            nc.vector.tensor_reduce(
                                out=msgt4[:],
                                in_=prod[:].rearrange("p (a i) -> p a i", i=d),
                                axis=mybir.AxisListType.X,
                                op=mybir.AluOpType.add,
                            )
                            for tt in range(4):
                                t = g0 + tt
                                tl = tl0 + tt
                                b = bases[t]
                                segs = []
                                lo = b
                                while lo < b + WIN:
                                    hi = min(b + WIN, (lo // 512 + 1) * 512)
                                    segs.append((lo, hi))
                                    lo = hi
                                for lhs in (
                                    msgt4[:, tt * d : (tt + 1) * d],
                                    gbuf[:, tl, d : 2 * d],
                                ):
                                    for (lo, hi) in segs:
                                        nc.tensor.matmul(
                                            aggT[:, lo:hi],
                                            lhs,
                                            hotv_sb[:, t, lo - b : hi - b],
                                            start=False,
                                            stop=False,
                                        )
                            continue_tiles = []
                        else:
                            continue_tiles = list(range(4))
                        for tt in continue_tiles:
                            t = g0 + tt
                            tl = t4 * 4 + tt  # tile within chunk
                            wp = ps_w.tile([128, D2], f32, tag="w", space="PSUM")
                            for wo in range(0, D2, 512):
                                wn = min(512, D2 - wo)
                                nc.tensor.matmul(
                                    wp[:, wo : wo + wn],
                                    hs[:, tt * 128 : (tt + 1) * 128],
                                    ew2pT_sb[L][:, wo : wo + wn],
                                    start=True,
                                    stop=True,
                                )
                            prod = pw_.tile([128, D2], bf16 if PROD_BF16 else f32, tag="prod")
                            xp0 = gbuf[:, tl, 0:d]
                            xp_b = bass.AP(
                                tensor=xp0.tensor,
                                offset=xp0.offset,
                                ap=[xp0.ap[0], [0, d], [1, d]],
                            )
                            nc.vector.tensor_tensor(
                                out=prod[:], in0=wp[:], in1=xp_b, op=mybir.AluOpType.mult
                            )
                            msgt = pw_.tile([128, d], f32, tag="msg")
                            nc.vector.tensor_reduce(
                                out=msgt[:],
                                in_=prod[:].rearrange("p (o i) -> p o i", i=d),
                                axis=mybir.AxisListType.X,
                                op=mybir.AluOpType.add,
                            )
                            b = bases[t]
                            segs = []
                            lo = b
                            while lo < b + WIN:
                                hi = min(b + WIN, (lo // 512 + 1) * 512)
                                segs.append((lo, hi))
                                lo = hi
                            for lhs in (msgt[:], gbuf[:, tl, d : 2 * d]):
                                for (lo, hi) in segs:
                                    nc.tensor.matmul(
                                        aggT[:, lo:hi],
                                        lhs,
                                        hotv_sb[:, t, lo - b : hi - b],
                                        start=False,
                                        stop=False,
                                    )

                # ---- root + bias into aggT ----
                for (o, n) in NCHUNK:
                    nc.tensor.matmul(
                        aggT[:, o : o + n],
                        wroot_sb[L][:],
                        xpxb_sb[0 : d + 1, o : o + n],
                        start=False,
                        stop=True,
                    )

                # ---- update: x_next = relu(conv) + xproj ----
                relu_sb = pw_.tile([d, NSP], f32, tag="relu")
                nc.scalar.activation(relu_sb[:], aggT[:], mybir.ActivationFunctionType.Relu)
                xT_next = px.tile([d + 1, NSP], f32, tag="xTn")
                nc.vector.memset(xT_next[:], 1.0)
                nc.vector.tensor_tensor(
                    out=xT_next[0:d, :],
                    in0=relu_sb[:],
                    in1=xpxb_sb[0:d, :],
                    op=mybir.AluOpType.add,
                )
                xT_cur = xT_next

            # ---- head: build x4 bf16 gather table ----
            d4 = DIMS[3]
            x4bf = pw_.tile([d4, NSP], bf16, tag="x4bf")
            nc.vector.tensor_copy(out=x4bf[:], in_=xT_cur[0:d4, :])
            for ch in range(10):
                tp = ps_h.tile([128, 512], bf16, tag="h", space="PSUM")
                nc.tensor.transpose(
                    out=tp[:, 0:d4],
                    in_=x4bf[:, ch * 128 : (ch + 1) * 128],
                    identity=id16b_sb[:],
                )
                nc.scalar.activation(
                    x4cat_sb[:, ch, 0:d4], tp[:, 0:d4], mybir.ActivationFunctionType.Copy
                )
            nc.sync.dma_start(
                out=x4in.ap().rearrange("(c p) w -> p c w", p=128), in_=x4cat_sb[:]
            )
            nc.gpsimd.collective_compute(
                "AllGather",
                mybir.AluOpType.bypass,
                replica_groups=[list(range(NCOR))],
                ins=[x4in[:]],
                outs=[x4ag[:]],
            )

            # ---- head compute ----
            for ck in range(4):
                gr = pg.tile([128, 1, TC * 128], bf16, tag="g")
                nc.gpsimd.dma_gather(
                    gr[:],
                    x4ag[:],
                    ridx_sb[:, ck * TC * 8 : (ck + 1) * TC * 8],
                    TC * 128,
                    TC * 128,
                    GWB,
                    transpose=True,
                    single_packet=False,
                )
                gc_ = pg.tile([128, 1, TC * 128], bf16, tag="g")
                nc.gpsimd.dma_gather(
                    gc_[:],
                    x4ag[:],
                    cidx_sb[:, ck * TC * 8 : (ck + 1) * TC * 8],
                    TC * 128,
                    TC * 128,
                    GWB,
                    transpose=True,
                    single_packet=False,
                )
                for s in range(TC * 128 // 512):
                    sl = slice(s * 512, (s + 1) * 512)
                    hh = ps_w.tile([16, 512], f32, tag="w", space="PSUM")
                    nc.tensor.matmul(hh[:], wh0a_sb[:], gr[0:17, 0, sl], start=True, stop=False)
                    nc.tensor.matmul(hh[:], wh0b_sb[:], gc_[0:17, 0, sl], start=False, stop=True)
                    hhs = pw_.tile([17, 512], f32, tag="hh")
                    nc.vector.memset(hhs[:], 1.0)
                    nc.scalar.activation(hhs[0:16, :], hh[:], mybir.ActivationFunctionType.Relu)
                    op = ps_w.tile([1, 512], f32, tag="w", space="PSUM")
                    nc.tensor.matmul(op[:], wh1_sb[:], hhs[:], start=True, stop=True)
                    oc = pw_.tile([1, 512], f32, tag="oc")
                    nc.scalar.activation(oc[:], op[:], mybir.ActivationFunctionType.Copy)
                    off = ck * TC * 128 + s * 512
                    nc.sync.dma_start(out=out_d[:, off : off + 512], in_=oc[:])

    nc.compile()
    return nc


_CACHE = {}


def kernel(**inputs):
    x = np.asarray(inputs["x"])
    edge_attr = np.asarray(inputs["edge_attr"])
    edge_index = np.asarray(inputs["edge_index"])
    params = []
    for i in range(4):
        params.append(tuple(
            np.asarray(inputs[k]) for k in (
                f"p_w{i}", f"p_b{i}", f"e1_w{i}", f"e1_b{i}",
                f"e2_w{i}", f"e2_b{i}", f"r_w{i}", f"c_b{i}",
            )
        ))
    m_w0 = np.asarray(inputs["m_w0"]); m_b0 = np.asarray(inputs["m_b0"])
    m_w1 = np.asarray(inputs["m_w1"]); m_b1 = np.asarray(inputs["m_b1"])

    T, bases, in_maps, keep = _host_prep(
        x, edge_attr, edge_index, params, m_w0, m_b0, m_w1, m_b1
    )

    key = (T, hashlib.sha1(edge_index.tobytes()).hexdigest())
    if key not in _CACHE:
        _CACHE[key] = _build_program(T, bases)
    nc = _CACHE[key]

    res = run_bass_kernel_spmd(nc, in_maps, core_ids=list(range(NCOR)))
    out = np.zeros(E, dtype=np.float32)
    for c in range(NCOR):
        oc = np.asarray(res.results[c]["out"]).reshape(-1)
        mask = keep[c] >= 0
        out[keep[c][mask]] = oc[mask]
    return out



# revision 8
# speedup vs baseline: 1.0905x; 1.0905x over previous
"""Trainium2 Bass kernel for DynamicGCN (4x NNConv + edge head), 8-core SPMD.

Strategy: edges are sharded by destination-node range (col) across 8 cores, so
the scatter-mean aggregation is core-local. Per layer, each core:
  node matmuls (own 1250-node shard, feature-major) -> PE-transpose to a
  node-major gather table -> AllGather across cores -> dma_gather per edge ->
  edge-NN (h, W) on PE -> per-edge bmm contraction on DVE (0-step broadcast
  multiply + grouped reduce) -> scatter-mean as PE matmuls into a resident
  PSUM accumulator using host-built inverse-degree one-hot blocks.
The final edge head uses bf16 transposed-gathers (feature-major) and two
accumulating matmuls.
"""

import os
import sys
import hashlib

import numpy as np

sys.path.insert(0, "/opt/trn_rl_repo")

import concourse.bass as bass
import concourse.bacc as bacc
import concourse.mybir as mybir
import concourse.tile as tile
from concourse.library_config import mlp as gpsimd_mlp_lib
from concourse.bass_utils import run_bass_kernel_spmd
from contextlib import ExitStack

# Problem dims (hardcoded per contract)
N = 10000
E = 160000
IN_C = 16
IN_E = 8
DIMS = [32, 16, 16, 16]
IND = [16, 32, 16, 16]
ENN_H = 128
EMLP_H = 16

NCOR = 8
PROD_BF16 = bool(int(os.environ.get("PROD_BF16", "1")))
# A/B profiling toggles (all default ON = full kernel)
_SKIP = set(os.environ.get("SKIP_STAGES", "").split(","))
NS = N // NCOR          # 1250 nodes per shard
NSP = 1280              # padded shard rows (10 chunks of 128)
NG = NCOR * NSP         # 10240 gather-table rows
WIN = 48                # scatter column window
GW = 64                 # gather payload floats (256B)
GWB = 128               # bf16 head payload elems (256B)

f32 = mybir.dt.float32
bf16 = mybir.dt.bfloat16
i16 = mybir.dt.int16


def _wrap_idx(idx):
    """int16 gather-index layout: [16, n/16] wrap, tiled to 128 partitions."""
    assert idx.size % 16 == 0
    w = idx.reshape(16, -1, order="F").astype(np.int16)
    return np.tile(w, (8, 1))


def _host_prep(x, edge_attr, edge_index, params, m_w0, m_b0, m_w1, m_b1):
    row = np.asarray(edge_index[0], dtype=np.int64)
    col = np.asarray(edge_index[1], dtype=np.int64)
    ea = np.asarray(edge_attr, dtype=np.float32)
    xf = np.asarray(x, dtype=np.float32)

    deg = np.bincount(col, minlength=N).astype(np.float32)
    inv_deg = 1.0 / np.maximum(deg, 1.0)

    # --- shard + sort + tile with static window schedule ---
    shard = col // NS
    per_core = []
    for c in range(NCOR):
        eids = np.nonzero(shard == c)[0]
        eids = eids[np.argsort(col[eids], kind="stable")]
        per_core.append(eids)

    def base_of(t):
        return max(0, min(8 * t - 20, NSP - WIN))

    # greedy fill under the static base schedule
    tiles_per_core = []
    for c in range(NCOR):
        eids = per_core[c]
        lc = col[eids] - NS * c
        tiles = []
        p = 0
        t = 0
        while p < len(eids):
            b = base_of(t)
            hi = b + WIN
            q = p
            while q < len(eids) and q - p < 128 and lc[q] < hi:
                q += 1
            assert q == p or lc[p] >= b, (
                f"window schedule violated: core {c} tile {t} col {lc[p]} < base {b}"
            )
            tiles.append(eids[p:q])
            p = q
            t += 1
        tiles_per_core.append(tiles)

    T = max(len(tl) for tl in tiles_per_core)
    T = 4 * ((T + 3) // 4)  # chunkable by 4
    EP = T * 128
    bases = [base_of(t) for t in range(T)]

    # --- per-core device arrays ---
    import ml_dtypes as _mld
    ea_sh = np.zeros((NCOR, 8, EP), dtype=_mld.bfloat16)
    hotv = np.zeros((NCOR, 128, T, WIN), dtype=np.float32)
    ridx = np.zeros((NCOR, 128, EP // 16), dtype=np.int16)
    cidx = np.zeros((NCOR, 128, EP // 16), dtype=np.int16)
    xT0 = np.zeros((NCOR, IND[0] + 1, NSP), dtype=np.float32)
    keep = []  # (positions-in-original-E, padded positions) per core
    for c in range(NCOR):
        gr = np.zeros(EP, dtype=np.int64)
        gc = np.zeros(EP, dtype=np.int64)
        orig = np.full(EP, -1, dtype=np.int64)
        for t, te in enumerate(tiles_per_core[c]):
            s = t * 128
            k = len(te)
            if k:
                ea_sh[c, :, s : s + k] = ea[te].T
                gr[s : s + k] = (row[te] // NS) * NSP + row[te] % NS
                gc[s : s + k] = (col[te] // NS) * NSP + col[te] % NS
                lcol = col[te] - NS * c
                hotv[c, np.arange(k), t, lcol - bases[t]] = inv_deg[col[te]]
                orig[s : s + k] = te
        ridx[c] = _wrap_idx(gr)
        cidx[c] = _wrap_idx(gc)
        xT0[c, :IND[0], :NS] = xf[NS * c : NS * (c + 1)].T
        xT0[c, IND[0], :NS] = 1.0
        keep.append(orig)

    import ml_dtypes
    # --- folded weights (shared across cores) ---
    Wnode, Wroot, ew1T, eb1, ew2pT = [], [], [], [], []
    for L in range(4):
        pw, pb, ew1, eb1_, ew2, eb2, rw, cb = params[L]
        d_in, d = IND[L], DIMS[L]
        B = eb2.reshape(d, d)
        xb_off = 64 if d == 32 else 32
        wn = np.zeros((d_in + 1, xb_off + d), dtype=np.float32)
        wn[:d_in, :d] = pw.T
        wn[d_in, :d] = pb
        wn[d_in, d] = 1.0
        wn[:d_in, xb_off:] = pw.T @ B
        wn[d_in, xb_off:] = pb @ B
        Wnode.append(wn)
        wr = np.zeros((d + 1, d), dtype=np.float32)
        wr[:d, :] = rw.T
        wr[d, :] = cb
        Wroot.append(wr)
        ew1T.append(np.ascontiguousarray(ew1.T).astype(ml_dtypes.bfloat16))
        eb1.append(eb1_.reshape(128, 1).astype(np.float32))
        # rhs[k, o*d+i] = ew2[i*d+o, k]
        ew2pT.append(
            np.ascontiguousarray(
                ew2.reshape(d, d, ENN_H).transpose(1, 0, 2).reshape(d * d, ENN_H).T
            ).astype(ml_dtypes.bfloat16)
        )
    wh0a = np.zeros((17, 16), dtype=np.float32)
    wh0a[:16] = m_w0.T
    wh0a[16] = m_b0
    wh0b = np.zeros((17, 16), dtype=np.float32)
    wh0b[:16] = m_w0.T
    wh1 = np.zeros((17, 1), dtype=np.float32)
    wh1[:16, 0] = m_w1[0]
    wh1[16, 0] = m_b1[0]
    id3 = np.tile(np.eye(32, dtype=np.float32), (3, 1))
    import ml_dtypes

    shared = {
        "wh0a": wh0a.astype(ml_dtypes.bfloat16),
        "wh0b": wh0b.astype(ml_dtypes.bfloat16),
        "wh1": wh1,
        "id3": id3,
        "ident16b": np.eye(16, dtype=np.float32).astype(ml_dtypes.bfloat16),
    }
    for L in range(4):
        shared[f"wnode{L}"] = Wnode[L]
        shared[f"wroot{L}"] = Wroot[L]
        shared[f"ew1T{L}"] = ew1T[L]
        shared[f"eb1{L}"] = eb1[L]
        shared[f"ew2pT{L}"] = ew2pT[L]

    in_maps = []
    for c in range(NCOR):
        m = {
            "ea_sh": ea_sh[c],
            "hotv": hotv[c],
            "ridx": ridx[c],
            "cidx": cidx[c],
            "xT0": xT0[c],
        }
        m.update(shared)
        in_maps.append(m)
    return T, bases, in_maps, keep


def _build_program(T, bases, repeat=1, skip_cc=False):
    EP = T * 128
    TC = T // 4  # tiles per gather chunk

    nc = bacc.Bacc(num_devices=NCOR)

    # I/O
    ea_d = nc.dram_tensor("ea_sh", [8, EP], bf16, kind="ExternalInput")
    hotv_d = nc.dram_tensor("hotv", [128, T, WIN], f32, kind="ExternalInput")
    ridx_d = nc.dram_tensor("ridx", [128, EP // 16], i16, kind="ExternalInput")
    cidx_d = nc.dram_tensor("cidx", [128, EP // 16], i16, kind="ExternalInput")
    xT0_d = nc.dram_tensor("xT0", [IND[0] + 1, NSP], f32, kind="ExternalInput")
    wh0a_d = nc.dram_tensor("wh0a", [17, 16], bf16, kind="ExternalInput")
    wh0b_d = nc.dram_tensor("wh0b", [17, 16], bf16, kind="ExternalInput")
    wh1_d = nc.dram_tensor("wh1", [17, 1], f32, kind="ExternalInput")
    id3_d = nc.dram_tensor("id3", [96, 32], f32, kind="ExternalInput")
    id16b_d = nc.dram_tensor("ident16b", [16, 16], bf16, kind="ExternalInput")
    wnode_d = [
        nc.dram_tensor(f"wnode{L}", [IND[L] + 1, (64 if DIMS[L] == 32 else 32) + DIMS[L]], f32, kind="ExternalInput")
        for L in range(4)
    ]
    wroot_d = [
        nc.dram_tensor(f"wroot{L}", [DIMS[L] + 1, DIMS[L]], f32, kind="ExternalInput")
        for L in range(4)
    ]
    ew1T_d = [nc.dram_tensor(f"ew1T{L}", [8, 128], bf16, kind="ExternalInput") for L in range(4)]
    eb1_d = [nc.dram_tensor(f"eb1{L}", [128, 1], f32, kind="ExternalInput") for L in range(4)]
    ew2pT_d = [
        nc.dram_tensor(f"ew2pT{L}", [128, DIMS[L] * DIMS[L]], bf16, kind="ExternalInput")
        for L in range(4)
    ]
    out_d = nc.dram_tensor("out", [1, EP], f32, kind="ExternalOutput")

    # internal DRAM
    xcat_in = [nc.dram_tensor(f"xcat_in{L}", [NSP, GW], f32) for L in range(4)]
    xag = [
        nc.dram_tensor(f"xag{L}", [NG, GW], f32, addr_space="Shared") for L in range(4)
    ]
    x4in = nc.dram_tensor("x4in", [NSP, GWB], bf16)
    x4ag = nc.dram_tensor("x4ag", [NG, GWB], bf16, addr_space="Shared")

    NCHUNK = [(0, 512), (512, 512), (1024, 256)]

    with tile.TileContext(nc) as tc, ExitStack() as ctx:
        cp = ctx.enter_context(tc.tile_pool(name="const", bufs=1))
        pg = ctx.enter_context(tc.tile_pool(name="gath", bufs=4))
        pw_ = ctx.enter_context(tc.tile_pool(name="work", bufs=2))
        ph4 = ctx.enter_context(tc.tile_pool(name="hpipe", bufs=6))
        px = ctx.enter_context(tc.tile_pool(name="xstate", bufs=2))
        ps_agg = ctx.enter_context(tc.tile_pool(name="psagg", bufs=1, space="PSUM"))
        ps_w = ctx.enter_context(tc.tile_pool(name="psw", bufs=2, space="PSUM"))
        ps_h = ctx.enter_context(tc.tile_pool(name="psh", bufs=1, space="PSUM"))

        nc.gpsimd.load_library(gpsimd_mlp_lib)

        # ---- resident constants ----
        hotv_sb = cp.tile([128, T, WIN], f32)
        nc.sync.dma_start(out=hotv_sb[:], in_=hotv_d[:])
        ridx_sb = cp.tile([128, EP // 16], i16)
        nc.sync.dma_start(out=ridx_sb[:], in_=ridx_d[:])
        cidx_sb = cp.tile([128, EP // 16], i16)
        nc.sync.dma_start(out=cidx_sb[:], in_=cidx_d[:])
        wh0a_sb = cp.tile([17, 16], bf16)
        nc.sync.dma_start(out=wh0a_sb[:], in_=wh0a_d[:])
        wh0b_sb = cp.tile([17, 16], bf16)
        nc.sync.dma_start(out=wh0b_sb[:], in_=wh0b_d[:])
        wh1_sb = cp.tile([17, 1], f32)
        nc.sync.dma_start(out=wh1_sb[:], in_=wh1_d[:])
        id3_sb = cp.tile([96, 32], f32)
        nc.sync.dma_start(out=id3_sb[:], in_=id3_d[:])
        id16b_sb = cp.tile([16, 16], bf16)
        nc.sync.dma_start(out=id16b_sb[:], in_=id16b_d[:])
        wnode_sb, wroot_sb, ew1T_sb, eb1_sb, ew2pT_sb = [], [], [], [], []
        for L in range(4):
            t_ = cp.tile([IND[L] + 1, (64 if DIMS[L] == 32 else 32) + DIMS[L]], f32, tag=f"wnode{L}")
            nc.sync.dma_start(out=t_[:], in_=wnode_d[L][:])
            wnode_sb.append(t_)
            t_ = cp.tile([DIMS[L] + 1, DIMS[L]], f32, tag=f"wroot{L}")
            nc.sync.dma_start(out=t_[:], in_=wroot_d[L][:])
            wroot_sb.append(t_)
            t_ = cp.tile([8, 128], bf16, tag=f"ew1T{L}")
            nc.sync.dma_start(out=t_[:], in_=ew1T_d[L][:])
            ew1T_sb.append(t_)
            t_ = cp.tile([128, 1], f32, tag=f"eb1{L}")
            nc.sync.dma_start(out=t_[:], in_=eb1_d[L][:])
            eb1_sb.append(t_)
            t_ = cp.tile([128, DIMS[L] * DIMS[L]], bf16, tag=f"ew2pT{L}")
            nc.sync.dma_start(out=t_[:], in_=ew2pT_d[L][:])
            ew2pT_sb.append(t_)
        zz = cp.tile([128, 512], f32)
        nc.vector.memset(zz[:], 0.0)
        xcat_sb = cp.tile([128, 10, GW], f32)
        nc.vector.memset(xcat_sb[:], 0.0)
        x4cat_sb = cp.tile([128, 10, GWB], bf16)
        nc.vector.memset(x4cat_sb[:], 0.0)
        nc.vector.memset(x4cat_sb[:, :, 16:17], 1.0)

        for _rep in range(repeat):
            # layer-0 x state
            xT_cur = px.tile([IND[0] + 1, NSP], f32, tag="xT0in")
            nc.sync.dma_start(out=xT_cur[:], in_=xT0_d[:])

            for L in range(4):
                d_in, d = IND[L], DIMS[L]
                A = d_in + 1
                XB = 64 if d == 32 else 32
                M2 = XB + d
                D2 = d * d

                # ---- node matmuls: xpxb = [xprojT; ones; xbT] ----
                xpxb_sb = pw_.tile([M2, NSP], f32, tag="xpxb")
                if "node" not in _SKIP:
                  for (o, n) in NCHUNK:
                    mm = ps_w.tile([M2, n], f32, tag="w", space="PSUM")
                    nc.tensor.matmul(
                        mm[:], wnode_sb[L][:], xT_cur[:A, o : o + n], start=True, stop=True
                    )
                    nc.scalar.activation(
                        xpxb_sb[:, o : o + n], mm[:], mybir.ActivationFunctionType.Copy
                    )

                # ---- transpose shard to node-major gather payload ----
                for ch in (range(10) if "node" not in _SKIP else []):
                    sl = slice(ch * 128, (ch + 1) * 128)
                    tp = ps_h.tile([128, 512], f32, tag="h", space="PSUM")
                    nc.tensor.transpose(
                        out=tp[:, 0:d], in_=xpxb_sb[0:d, sl], identity=id3_sb[0:d, 0:d]
                    )
                    nc.scalar.activation(
                        xcat_sb[:, ch, 0:d], tp[:, 0:d], mybir.ActivationFunctionType.Copy
                    )
                    tp2 = ps_h.tile([128, 512], f32, tag="h", space="PSUM")
                    nc.tensor.transpose(
                        out=tp2[:, 0:d],
                        in_=xpxb_sb[XB:M2, sl],
                        identity=id3_sb[XB : XB + d, 0:d],
                    )
                    nc.scalar.activation(
                        xcat_sb[:, ch, d : 2 * d],
                        tp2[:, 0:d],
                        mybir.ActivationFunctionType.Copy,
                    )
                if "node" not in _SKIP:
                    nc.sync.dma_start(
                        out=xcat_in[L].ap().rearrange("(c p) w -> p c w", p=128),
                        in_=xcat_sb[:],
                    )
                if "ag" not in _SKIP:
                    nc.gpsimd.collective_compute(
                        "AllGather",
                        mybir.AluOpType.bypass,
                        replica_groups=[list(range(NCOR))],
                        ins=[xcat_in[L][:]],
                        outs=[xag[L][:]],
                    )

                # ---- zero-init aggT ----
                aggT = ps_agg.tile([d, NSP], f32, tag="agg", space="PSUM")
                for (o, n) in NCHUNK:
                    nc.tensor.matmul(
                        aggT[:, o : o + n], zz[:, 0:d], zz[:, 0:n], start=True, stop=False
                    )

                # ---- edge phase ----
                for ck in range(4):
                    gbuf = pg.tile([128, TC, GW], f32, tag="g")
                    if "gather" not in _SKIP:
                        nc.gpsimd.dma_gather(
                            gbuf[:],
                            xag[L][:],
                            ridx_sb[:, ck * TC * 8 : (ck + 1) * TC * 8],
                            TC * 128,
                            TC * 128,
                            GW,
                            single_packet=False,
                        )
                    for t4 in range(TC // 4):
                        g0 = ck * TC + t4 * 4  # global tile of first in group
                        es = g0 * 128  # edge offset
                        ea_g = ph4.tile([8, 512], bf16, tag="ea")
                        if "ea" not in _SKIP:
                            nc.sync.dma_start(out=ea_g[:], in_=ea_d[:, es : es + 512])
                        hs = ph4.tile([128, 512], bf16, tag="hs")
                        if "h" not in _SKIP:
                            hp = ps_h.tile([128, 512], f32, tag="h", space="PSUM")
                            nc.tensor.matmul(
                                hp[:], ew1T_sb[L][:], ea_g[:], start=True, stop=True
                            )
                            nc.scalar.activation(
                                hs[:], hp[:], mybir.ActivationFunctionType.Relu, bias=eb1_sb[L][:]
                            )
                        if d == 16:
                            # batched: 4 tiles share one [128,1024] W tile and
                            # one DVE mult+reduce (amortizes per-op overhead)
                            tl0 = t4 * 4
                            wp = ps_w.tile([128, 1024], f32, tag="w", space="PSUM")
                            if "w" not in _SKIP:
                              for tt in range(4):
                                nc.tensor.matmul(
                                    wp[:, tt * 256 : (tt + 1) * 256],
                                    hs[:, tt * 128 : (tt + 1) * 128],
                                    ew2pT_sb[L][:],
                                    start=True,
                                    stop=True,
                                )
                            prod = pw_.tile([128, 1024], bf16 if PROD_BF16 else f32, tag="prod")
                            msgt4 = pw_.tile([128, 64], f32, tag="msg")
                            if "dve" not in _SKIP:
                              xp0 = gbuf[:, tl0, 0:d]
                              xp_b = bass.AP(
                                tensor=xp0.tensor,
                                offset=xp0.offset,
                                ap=[xp0.ap[0], [GW, 4], [0, d], [1, d]],
                              )
                              nc.vector.tensor_tensor(
                                out=prod[:], in0=wp[:], in1=xp_b, op=mybir.AluOpType.mult
                              )
                              nc.vector.tensor_reduce(
                                out=msgt4[:],
                                in_=prod[:].rearrange("p (a i) -> p a i", i=d),
                                axis=mybir.AxisListType.X,
                                op=mybir.AluOpType.add,
                              )
                            for tt in (range(4) if "scatter" not in _SKIP else []):
                                t = g0 + tt
                                tl = tl0 + tt
                                b = bases[t]
                                segs = []
                                lo = b
                                while lo < b + WIN:
                                    hi = min(b + WIN, (lo // 512 + 1) * 512)
                                    segs.append((lo, hi))
                                    lo = hi
                                for lhs in (
                                    msgt4[:, tt * d : (tt + 1) * d],
                                    gbuf[:, tl, d : 2 * d],
                                ):
                                    for (lo, hi) in segs:
                                        nc.tensor.matmul(
                                            aggT[:, lo:hi],
                                            lhs,
                                            hotv_sb[:, t, lo - b : hi - b],
                                            start=False,
                                            stop=False,
                                        )
                            continue_tiles = []
                        else:
                            continue_tiles = list(range(4))
                        for tt in continue_tiles:
                            t = g0 + tt
                            tl = t4 * 4 + tt  # tile within chunk
                            wp = ps_w.tile([128, D2], f32, tag="w", space="PSUM")
                            if "w" not in _SKIP:
                              for wo in range(0, D2, 512):
                                wn = min(512, D2 - wo)
                                nc.tensor.matmul(
                                    wp[:, wo : wo + wn],
                                    hs[:, tt * 128 : (tt + 1) * 128],
                                    ew2pT_sb[L][:, wo : wo + wn],
                                    start=True,
                                    stop=True,
                                )
                            prod = pw_.tile([128, D2], bf16 if PROD_BF16 else f32, tag="prod")
                            msgt = pw_.tile([128, d], f32, tag="msg")
                            if "dve" not in _SKIP:
                              xp0 = gbuf[:, tl, 0:d]
                              xp_b = bass.AP(
                                tensor=xp0.tensor,
                                offset=xp0.offset,
                                ap=[xp0.ap[0], [0, d], [1, d]],
                              )
                              nc.vector.tensor_tensor(
                                out=prod[:], in0=wp[:], in1=xp_b, op=mybir.AluOpType.mult
                              )
                              nc.vector.tensor_reduce(
                                out=msgt[:],
                                in_=prod[:].rearrange("p (o i) -> p o i", i=d),
                                axis=mybir.AxisListType.X,
                                op=mybir.AluOpType.add,
                              )
                            if "scatter" not in _SKIP:
                              b = bases[t]
                              segs = []
                              lo = b
                              while lo < b + WIN:
                                hi = min(b + WIN, (lo // 512 + 1) * 512)
                                segs.append((lo, hi))
                                lo = hi
                              for lhs in (msgt[:], gbuf[:, tl, d : 2 * d]):
                                for (lo, hi) in segs:
                                    nc.tensor.matmul(
                                        aggT[:, lo:hi],
                                        lhs,
                                        hotv_sb[:, t, lo - b : hi - b],
                                        start=False,
                                        stop=False,
                                    )

                # ---- root + bias into aggT ----
                for (o, n) in NCHUNK:
                    nc.tensor.matmul(
                        aggT[:, o : o + n],
                        wroot_sb[L][:],
                        xpxb_sb[0 : d + 1, o : o + n],
                        start=False,
                        stop=True,
                    )

                # ---- update: x_next = relu(conv) + xproj ----
                relu_sb = pw_.tile([d, NSP], f32, tag="relu")
                nc.scalar.activation(relu_sb[:], aggT[:], mybir.ActivationFunctionType.Relu)
                xT_next = px.tile([d + 1, NSP], f32, tag="xTn")
                nc.vector.memset(xT_next[:], 1.0)
                nc.vector.tensor_tensor(
                    out=xT_next[0:d, :],
                    in0=relu_sb[:],
                    in1=xpxb_sb[0:d, :],
                    op=mybir.AluOpType.add,
                )
                xT_cur = xT_next

            # ---- head: build x4 bf16 gather table ----
            d4 = DIMS[3]
            x4bf = pw_.tile([d4, NSP], bf16, tag="x4bf")
            nc.vector.tensor_copy(out=x4bf[:], in_=xT_cur[0:d4, :])
            for ch in range(10):
                tp = ps_h.tile([128, 512], bf16, tag="h", space="PSUM")
                nc.tensor.transpose(
                    out=tp[:, 0:d4],
                    in_=x4bf[:, ch * 128 : (ch + 1) * 128],
                    identity=id16b_sb[:],
                )
                nc.scalar.activation(
                    x4cat_sb[:, ch, 0:d4], tp[:, 0:d4], mybir.ActivationFunctionType.Copy
                )
            nc.sync.dma_start(
                out=x4in.ap().rearrange("(c p) w -> p c w", p=128), in_=x4cat_sb[:]
            )
            if "ag" not in _SKIP:
                nc.gpsimd.collective_compute(
                    "AllGather",
                    mybir.AluOpType.bypass,
                    replica_groups=[list(range(NCOR))],
                    ins=[x4in[:]],
                    outs=[x4ag[:]],
                )

            # ---- head compute ----
            for ck in (range(4) if "head" not in _SKIP else []):
                gr = pg.tile([128, 1, TC * 128], bf16, tag="g")
                if "gather" not in _SKIP:
                    nc.gpsimd.dma_gather(
                        gr[:],
                        x4ag[:],
                        ridx_sb[:, ck * TC * 8 : (ck + 1) * TC * 8],
                        TC * 128,
                        TC * 128,
                        GWB,
                        transpose=True,
                        single_packet=False,
                    )
                gc_ = pg.tile([128, 1, TC * 128], bf16, tag="g")
                if "gather" not in _SKIP:
                    nc.gpsimd.dma_gather(
                        gc_[:],
                        x4ag[:],
                        cidx_sb[:, ck * TC * 8 : (ck + 1) * TC * 8],
                        TC * 128,
                        TC * 128,
                        GWB,
                        transpose=True,
                        single_packet=False,
                    )
                for s in range(TC * 128 // 512):
                    sl = slice(s * 512, (s + 1) * 512)
                    hh = ps_w.tile([16, 512], f32, tag="w", space="PSUM")
                    nc.tensor.matmul(hh[:], wh0a_sb[:], gr[0:17, 0, sl], start=True, stop=False)
                    nc.tensor.matmul(hh[:], wh0b_sb[:], gc_[0:17, 0, sl], start=False, stop=True)
                    hhs = pw_.tile([17, 512], f32, tag="hh")
                    nc.vector.memset(hhs[:], 1.0)
                    nc.scalar.activation(hhs[0:16, :], hh[:], mybir.ActivationFunctionType.Relu)
                    op = ps_w.tile([1, 512], f32, tag="w", space="PSUM")
                    nc.tensor.matmul(op[:], wh1_sb[:], hhs[:], start=True, stop=True)
                    oc = pw_.tile([1, 512], f32, tag="oc")
                    nc.scalar.activation(oc[:], op[:], mybir.ActivationFunctionType.Copy)
                    off = ck * TC * 128 + s * 512
                    nc.sync.dma_start(out=out_d[:, off : off + 512], in_=oc[:])

    nc.compile()
    return nc


_CACHE = {}


def kernel(**inputs):
    x = np.asarray(inputs["x"])
    edge_attr = np.asarray(inputs["edge_attr"])
    edge_index = np.asarray(inputs["edge_index"])
    params = []
    for i in range(4):
        params.append(tuple(
            np.asarray(inputs[k]) for k in (
                f"p_w{i}", f"p_b{i}", f"e1_w{i}", f"e1_b{i}",
                f"e2_w{i}", f"e2_b{i}", f"r_w{i}", f"c_b{i}",
            )
        ))
    m_w0 = np.asarray(inputs["m_w0"]); m_b0 = np.asarray(inputs["m_b0"])
    m_w1 = np.asarray(inputs["m_w1"]); m_b1 = np.asarray(inputs["m_b1"])

    T, bases, in_maps, keep = _host_prep(
        x, edge_attr, edge_index, params, m_w0, m_b0, m_w1, m_b1
    )

    key = (T, hashlib.sha1(edge_index.tobytes()).hexdigest())
    if key not in _CACHE:
        _CACHE[key] = _build_program(T, bases)
    nc = _CACHE[key]

    res = run_bass_kernel_spmd(nc, in_maps, core_ids=list(range(NCOR)))
    out = np.zeros(E, dtype=np.float32)
    for c in range(NCOR):
        oc = np.asarray(res.results[c]["out"]).reshape(-1)
        mask = keep[c] >= 0
        out[keep[c][mask]] = oc[mask]
    return out



# revision 9
# speedup vs baseline: 1.3687x; 1.2552x over previous
"""Trainium2 Bass kernel for DynamicGCN (4x NNConv + edge head), 8-core SPMD.

Strategy: edges are sharded by destination-node range (col) across 8 cores, so
the scatter-mean aggregation is core-local. Per layer, each core:
  node matmuls (own 1250-node shard, feature-major) -> PE-transpose to a
  node-major gather table -> AllGather across cores -> dma_gather per edge ->
  edge-NN (h, W) on PE -> per-edge bmm contraction on DVE (0-step broadcast
  multiply + grouped reduce) -> scatter-mean as PE matmuls into a resident
  PSUM accumulator using host-built inverse-degree one-hot blocks.
The final edge head uses bf16 transposed-gathers (feature-major) and two
accumulating matmuls.
"""

import os
import sys
import hashlib

import numpy as np

sys.path.insert(0, "/opt/trn_rl_repo")

import concourse.bass as bass
import concourse.bacc as bacc
import concourse.mybir as mybir
import concourse.tile as tile
from concourse.library_config import mlp as gpsimd_mlp_lib
from concourse.bass_utils import run_bass_kernel_spmd
from contextlib import ExitStack

# Problem dims (hardcoded per contract)
N = 10000
E = 160000
IN_C = 16
IN_E = 8
DIMS = [32, 16, 16, 16]
IND = [16, 32, 16, 16]
ENN_H = 128
EMLP_H = 16

NCOR = 8
PROD_BF16 = bool(int(os.environ.get("PROD_BF16", "1")))
# A/B profiling toggles (all default ON = full kernel)
_SKIP = set(os.environ.get("SKIP_STAGES", "").split(","))
NS = N // NCOR          # 1250 nodes per shard
NSP = 1280              # padded shard rows (10 chunks of 128)
NG = NCOR * NSP         # 10240 gather-table rows
WIN = 48                # scatter column window
GW = 64                 # gather payload floats (256B)
GWB = 128               # bf16 head payload elems (256B)

f32 = mybir.dt.float32
bf16 = mybir.dt.bfloat16
i16 = mybir.dt.int16


def _wrap_idx(idx):
    """int16 gather-index layout: [16, n/16] wrap, tiled to 128 partitions."""
    assert idx.size % 16 == 0
    w = idx.reshape(16, -1, order="F").astype(np.int16)
    return np.tile(w, (8, 1))


def _host_prep(x, edge_attr, edge_index, params, m_w0, m_b0, m_w1, m_b1):
    row = np.asarray(edge_index[0], dtype=np.int64)
    col = np.asarray(edge_index[1], dtype=np.int64)
    ea = np.asarray(edge_attr, dtype=np.float32)
    xf = np.asarray(x, dtype=np.float32)

    deg = np.bincount(col, minlength=N).astype(np.float32)
    inv_deg = 1.0 / np.maximum(deg, 1.0)

    # --- shard + sort + tile with static window schedule ---
    shard = col // NS
    per_core = []
    for c in range(NCOR):
        eids = np.nonzero(shard == c)[0]
        eids = eids[np.argsort(col[eids], kind="stable")]
        per_core.append(eids)

    def base_of(t):
        return max(0, min(8 * t - 20, NSP - WIN))

    # greedy fill under the static base schedule
    tiles_per_core = []
    for c in range(NCOR):
        eids = per_core[c]
        lc = col[eids] - NS * c
        tiles = []
        p = 0
        t = 0
        while p < len(eids):
            b = base_of(t)
            hi = b + WIN
            q = p
            while q < len(eids) and q - p < 128 and lc[q] < hi:
                q += 1
            assert q == p or lc[p] >= b, (
                f"window schedule violated: core {c} tile {t} col {lc[p]} < base {b}"
            )
            tiles.append(eids[p:q])
            p = q
            t += 1
        tiles_per_core.append(tiles)

    T = max(len(tl) for tl in tiles_per_core)
    T = 4 * ((T + 3) // 4)  # chunkable by 4
    EP = T * 128
    bases = [base_of(t) for t in range(T)]

    # --- per-core device arrays ---
    import ml_dtypes as _mld
    ea_sh = np.zeros((NCOR, 8, EP), dtype=_mld.bfloat16)
    hotv = np.zeros((NCOR, 128, T, WIN), dtype=np.float32)
    ridx = np.zeros((NCOR, 128, EP // 16), dtype=np.int16)
    cidx = np.zeros((NCOR, 128, EP // 16), dtype=np.int16)
    xT0 = np.zeros((NCOR, IND[0] + 1, NSP), dtype=np.float32)
    keep = []  # (positions-in-original-E, padded positions) per core
    for c in range(NCOR):
        gr = np.zeros(EP, dtype=np.int64)
        gc = np.zeros(EP, dtype=np.int64)
        orig = np.full(EP, -1, dtype=np.int64)
        for t, te in enumerate(tiles_per_core[c]):
            s = t * 128
            k = len(te)
            if k:
                ea_sh[c, :, s : s + k] = ea[te].T
                gr[s : s + k] = (row[te] // NS) * NSP + row[te] % NS
                gc[s : s + k] = (col[te] // NS) * NSP + col[te] % NS
                lcol = col[te] - NS * c
                hotv[c, np.arange(k), t, lcol - bases[t]] = inv_deg[col[te]]
                orig[s : s + k] = te
        ridx[c] = _wrap_idx(gr)
        cidx[c] = _wrap_idx(gc)
        xT0[c, :IND[0], :NS] = xf[NS * c : NS * (c + 1)].T
        xT0[c, IND[0], :NS] = 1.0
        keep.append(orig)

    import ml_dtypes
    # --- folded weights (shared across cores) ---
    Wnode, Wroot, ew1T, eb1, ew2pT = [], [], [], [], []
    for L in range(4):
        pw, pb, ew1, eb1_, ew2, eb2, rw, cb = params[L]
        d_in, d = IND[L], DIMS[L]
        B = eb2.reshape(d, d)
        xb_off = 64 if d == 32 else 32
        wn = np.zeros((d_in + 1, xb_off + d), dtype=np.float32)
        wn[:d_in, :d] = pw.T
        wn[d_in, :d] = pb
        wn[d_in, d] = 1.0
        wn[:d_in, xb_off:] = pw.T @ B
        wn[d_in, xb_off:] = pb @ B
        Wnode.append(wn)
        wr = np.zeros((d + 1, d), dtype=np.float32)
        wr[:d, :] = rw.T
        wr[d, :] = cb
        Wroot.append(wr)
        ew1T.append(np.ascontiguousarray(ew1.T).astype(ml_dtypes.bfloat16))
        eb1.append(eb1_.reshape(128, 1).astype(np.float32))
        # rhs[k, o*d+i] = ew2[i*d+o, k]
        ew2pT.append(
            np.ascontiguousarray(
                ew2.reshape(d, d, ENN_H).transpose(1, 0, 2).reshape(d * d, ENN_H).T
            ).astype(ml_dtypes.bfloat16)
        )
    wh0a = np.zeros((17, 16), dtype=np.float32)
    wh0a[:16] = m_w0.T
    wh0a[16] = m_b0
    wh0b = np.zeros((17, 16), dtype=np.float32)
    wh0b[:16] = m_w0.T
    wh1 = np.zeros((17, 1), dtype=np.float32)
    wh1[:16, 0] = m_w1[0]
    wh1[16, 0] = m_b1[0]
    id3 = np.tile(np.eye(32, dtype=np.float32), (3, 1))
    import ml_dtypes

    shared = {
        "wh0a": wh0a.astype(ml_dtypes.bfloat16),
        "wh0b": wh0b.astype(ml_dtypes.bfloat16),
        "wh1": wh1,
        "id3": id3,
        "ident16b": np.eye(16, dtype=np.float32).astype(ml_dtypes.bfloat16),
    }
    for L in range(4):
        shared[f"wnode{L}"] = Wnode[L]
        shared[f"wroot{L}"] = Wroot[L]
        shared[f"ew1T{L}"] = ew1T[L]
        shared[f"eb1{L}"] = eb1[L]
        shared[f"ew2pT{L}"] = ew2pT[L]

    in_maps = []
    for c in range(NCOR):
        m = {
            "ea_sh": ea_sh[c],
            "hotv": hotv[c],
            "ridx": ridx[c],
            "cidx": cidx[c],
            "xT0": xT0[c],
        }
        m.update(shared)
        in_maps.append(m)
    return T, bases, in_maps, keep


def _build_program(T, bases, repeat=1, skip_cc=False):
    EP = T * 128
    TC = T // 4  # tiles per gather chunk

    nc = bacc.Bacc(num_devices=NCOR)

    # I/O
    ea_d = nc.dram_tensor("ea_sh", [8, EP], bf16, kind="ExternalInput")
    hotv_d = nc.dram_tensor("hotv", [128, T, WIN], f32, kind="ExternalInput")
    ridx_d = nc.dram_tensor("ridx", [128, EP // 16], i16, kind="ExternalInput")
    cidx_d = nc.dram_tensor("cidx", [128, EP // 16], i16, kind="ExternalInput")
    xT0_d = nc.dram_tensor("xT0", [IND[0] + 1, NSP], f32, kind="ExternalInput")
    wh0a_d = nc.dram_tensor("wh0a", [17, 16], bf16, kind="ExternalInput")
    wh0b_d = nc.dram_tensor("wh0b", [17, 16], bf16, kind="ExternalInput")
    wh1_d = nc.dram_tensor("wh1", [17, 1], f32, kind="ExternalInput")
    id3_d = nc.dram_tensor("id3", [96, 32], f32, kind="ExternalInput")
    id16b_d = nc.dram_tensor("ident16b", [16, 16], bf16, kind="ExternalInput")
    wnode_d = [
        nc.dram_tensor(f"wnode{L}", [IND[L] + 1, (64 if DIMS[L] == 32 else 32) + DIMS[L]], f32, kind="ExternalInput")
        for L in range(4)
    ]
    wroot_d = [
        nc.dram_tensor(f"wroot{L}", [DIMS[L] + 1, DIMS[L]], f32, kind="ExternalInput")
        for L in range(4)
    ]
    ew1T_d = [nc.dram_tensor(f"ew1T{L}", [8, 128], bf16, kind="ExternalInput") for L in range(4)]
    eb1_d = [nc.dram_tensor(f"eb1{L}", [128, 1], f32, kind="ExternalInput") for L in range(4)]
    ew2pT_d = [
        nc.dram_tensor(f"ew2pT{L}", [128, DIMS[L] * DIMS[L]], bf16, kind="ExternalInput")
        for L in range(4)
    ]
    out_d = nc.dram_tensor("out", [1, EP], f32, kind="ExternalOutput")

    # internal DRAM
    xcat_in = [nc.dram_tensor(f"xcat_in{L}", [NSP, GW], f32) for L in range(4)]
    xag = [
        nc.dram_tensor(f"xag{L}", [NG, GW], f32, addr_space="Shared") for L in range(4)
    ]
    x4in = nc.dram_tensor("x4in", [NSP, GWB], bf16)
    x4ag = nc.dram_tensor("x4ag", [NG, GWB], bf16, addr_space="Shared")

    NCHUNK = [(0, 512), (512, 512), (1024, 256)]

    with tile.TileContext(nc) as tc, ExitStack() as ctx:
        cp = ctx.enter_context(tc.tile_pool(name="const", bufs=1))
        pg = ctx.enter_context(tc.tile_pool(name="gath", bufs=4))
        pw_ = ctx.enter_context(tc.tile_pool(name="work", bufs=2))
        ph4 = ctx.enter_context(tc.tile_pool(name="hpipe", bufs=6))
        px = ctx.enter_context(tc.tile_pool(name="xstate", bufs=2))
        ps_agg = ctx.enter_context(tc.tile_pool(name="psagg", bufs=1, space="PSUM"))
        ps_w = ctx.enter_context(tc.tile_pool(name="psw", bufs=2, space="PSUM"))
        ps_h = ctx.enter_context(tc.tile_pool(name="psh", bufs=1, space="PSUM"))

        nc.gpsimd.load_library(gpsimd_mlp_lib)

        # ---- resident constants ----
        hotv_sb = cp.tile([128, T, WIN], f32)
        nc.sync.dma_start(out=hotv_sb[:], in_=hotv_d[:])
        ridx_sb = cp.tile([128, EP // 16], i16)
        nc.sync.dma_start(out=ridx_sb[:], in_=ridx_d[:])
        cidx_sb = cp.tile([128, EP // 16], i16)
        nc.sync.dma_start(out=cidx_sb[:], in_=cidx_d[:])
        wh0a_sb = cp.tile([17, 16], bf16)
        nc.sync.dma_start(out=wh0a_sb[:], in_=wh0a_d[:])
        wh0b_sb = cp.tile([17, 16], bf16)
        nc.sync.dma_start(out=wh0b_sb[:], in_=wh0b_d[:])
        wh1_sb = cp.tile([17, 1], f32)
        nc.sync.dma_start(out=wh1_sb[:], in_=wh1_d[:])
        id3_sb = cp.tile([96, 32], f32)
        nc.sync.dma_start(out=id3_sb[:], in_=id3_d[:])
        id16b_sb = cp.tile([16, 16], bf16)
        nc.sync.dma_start(out=id16b_sb[:], in_=id16b_d[:])
        wnode_sb, wroot_sb, ew1T_sb, eb1_sb, ew2pT_sb = [], [], [], [], []
        for L in range(4):
            t_ = cp.tile([IND[L] + 1, (64 if DIMS[L] == 32 else 32) + DIMS[L]], f32, tag=f"wnode{L}")
            nc.sync.dma_start(out=t_[:], in_=wnode_d[L][:])
            wnode_sb.append(t_)
            t_ = cp.tile([DIMS[L] + 1, DIMS[L]], f32, tag=f"wroot{L}")
            nc.sync.dma_start(out=t_[:], in_=wroot_d[L][:])
            wroot_sb.append(t_)
            t_ = cp.tile([8, 128], bf16, tag=f"ew1T{L}")
            nc.sync.dma_start(out=t_[:], in_=ew1T_d[L][:])
            ew1T_sb.append(t_)
            t_ = cp.tile([128, 1], f32, tag=f"eb1{L}")
            nc.sync.dma_start(out=t_[:], in_=eb1_d[L][:])
            eb1_sb.append(t_)
            t_ = cp.tile([128, DIMS[L] * DIMS[L]], bf16, tag=f"ew2pT{L}")
            nc.sync.dma_start(out=t_[:], in_=ew2pT_d[L][:])
            ew2pT_sb.append(t_)
        zz = cp.tile([128, 512], f32)
        nc.vector.memset(zz[:], 0.0)
        xcat_sb = cp.tile([128, 10, GW], f32)
        nc.vector.memset(xcat_sb[:], 0.0)
        x4cat_sb = cp.tile([128, 10, GWB], bf16)
        nc.vector.memset(x4cat_sb[:], 0.0)
        nc.vector.memset(x4cat_sb[:, :, 16:17], 1.0)

        for _rep in range(repeat):
            # layer-0 x state
            xT_cur = px.tile([IND[0] + 1, NSP], f32, tag="xT0in")
            nc.sync.dma_start(out=xT_cur[:], in_=xT0_d[:])

            for L in range(4):
                d_in, d = IND[L], DIMS[L]
                A = d_in + 1
                XB = 64 if d == 32 else 32
                M2 = XB + d
                D2 = d * d

                # ---- node matmuls: xpxb = [xprojT; ones; xbT] ----
                xpxb_sb = pw_.tile([M2, NSP], f32, tag="xpxb")
                if "node" in _SKIP:
                    nc.vector.memset(xpxb_sb[:, 0:2], 0.0)
                if "node" not in _SKIP:
                  for (o, n) in NCHUNK:
                    mm = ps_w.tile([M2, n], f32, tag="w", space="PSUM")
                    nc.tensor.matmul(
                        mm[:], wnode_sb[L][:], xT_cur[:A, o : o + n], start=True, stop=True
                    )
                    nc.scalar.activation(
                        xpxb_sb[:, o : o + n], mm[:], mybir.ActivationFunctionType.Copy
                    )

                # ---- transpose shard to node-major gather payload ----
                for ch in (range(10) if "node" not in _SKIP else []):
                    sl = slice(ch * 128, (ch + 1) * 128)
                    tp = ps_h.tile([128, 512], f32, tag="h", space="PSUM")
                    nc.tensor.transpose(
                        out=tp[:, 0:d], in_=xpxb_sb[0:d, sl], identity=id3_sb[0:d, 0:d]
                    )
                    nc.scalar.activation(
                        xcat_sb[:, ch, 0:d], tp[:, 0:d], mybir.ActivationFunctionType.Copy
                    )
                    tp2 = ps_h.tile([128, 512], f32, tag="h", space="PSUM")
                    nc.tensor.transpose(
                        out=tp2[:, 0:d],
                        in_=xpxb_sb[XB:M2, sl],
                        identity=id3_sb[XB : XB + d, 0:d],
                    )
                    nc.scalar.activation(
                        xcat_sb[:, ch, d : 2 * d],
                        tp2[:, 0:d],
                        mybir.ActivationFunctionType.Copy,
                    )
                if "node" not in _SKIP:
                    nc.sync.dma_start(
                        out=xcat_in[L].ap().rearrange("(c p) w -> p c w", p=128),
                        in_=xcat_sb[:],
                    )
                if "ag" not in _SKIP:
                    nc.gpsimd.collective_compute(
                        "AllGather",
                        mybir.AluOpType.bypass,
                        replica_groups=[list(range(NCOR))],
                        ins=[xcat_in[L][:]],
                        outs=[xag[L][:]],
                    )

                # ---- zero-init aggT ----
                aggT = ps_agg.tile([d, NSP], f32, tag="agg", space="PSUM")
                for (o, n) in NCHUNK:
                    nc.tensor.matmul(
                        aggT[:, o : o + n], zz[:, 0:d], zz[:, 0:n], start=True, stop=False
                    )

                # ---- edge phase ----
                for ck in range(4):
                    gbuf = pg.tile([128, TC, GW], f32, tag="g")
                    if "gather" in _SKIP:
                        nc.vector.memset(gbuf[:, 0:1, 0:2], 0.0)
                    if "gather" not in _SKIP:
                        nc.gpsimd.dma_gather(
                            gbuf[:],
                            xag[L][:],
                            ridx_sb[:, ck * TC * 8 : (ck + 1) * TC * 8],
                            TC * 128,
                            TC * 128,
                            GW,
                            single_packet=False,
                        )
                    for t4 in range(TC // 4):
                        g0 = ck * TC + t4 * 4  # global tile of first in group
                        es = g0 * 128  # edge offset
                        ea_g = ph4.tile([8, 512], bf16, tag="ea")
                        if "ea" in _SKIP:
                            nc.vector.memset(ea_g[:, 0:2], 0.0)
                        if "ea" not in _SKIP:
                            nc.sync.dma_start(out=ea_g[:], in_=ea_d[:, es : es + 512])
                        hs = ph4.tile([128, 512], bf16, tag="hs")
                        if "h" in _SKIP:
                            nc.vector.memset(hs[:, 0:2], 0.0)
                        if "h" not in _SKIP:
                            hp = ps_h.tile([128, 512], f32, tag="h", space="PSUM")
                            nc.tensor.matmul(
                                hp[:], ew1T_sb[L][:], ea_g[:], start=True, stop=True
                            )
                            nc.scalar.activation(
                                hs[:], hp[:], mybir.ActivationFunctionType.Relu, bias=eb1_sb[L][:]
                            )
                        if d == 16:
                            # batched: 4 tiles share one [128,1024] W tile and
                            # one DVE mult+reduce (amortizes per-op overhead)
                            tl0 = t4 * 4
                            wp = ps_w.tile([128, 1024], f32, tag="w", space="PSUM")
                            if "w" in _SKIP:
                                nc.vector.memset(wp[:, 0:2], 0.0)
                            if "w" not in _SKIP:
                              for tt in range(4):
                                nc.tensor.matmul(
                                    wp[:, tt * 256 : (tt + 1) * 256],
                                    hs[:, tt * 128 : (tt + 1) * 128],
                                    ew2pT_sb[L][:],
                                    start=True,
                                    stop=True,
                                )
                            prod = pw_.tile([128, 1024], bf16 if PROD_BF16 else f32, tag="prod")
                            msgt4 = pw_.tile([128, 64], f32, tag="msg")
                            if "dve" in _SKIP:
                                nc.vector.memset(prod[:, 0:2], 0.0)
                                nc.vector.memset(msgt4[:, 0:2], 0.0)
                            if "dve" not in _SKIP:
                              xp0 = gbuf[:, tl0, 0:d]
                              xp_b = bass.AP(
                                tensor=xp0.tensor,
                                offset=xp0.offset,
                                ap=[xp0.ap[0], [GW, 4], [0, d], [1, d]],
                              )
                              nc.vector.tensor_tensor(
                                out=prod[:], in0=wp[:], in1=xp_b, op=mybir.AluOpType.mult
                              )
                              nc.vector.tensor_reduce(
                                out=msgt4[:],
                                in_=prod[:].rearrange("p (a i) -> p a i", i=d),
                                axis=mybir.AxisListType.X,
                                op=mybir.AluOpType.add,
                              )
                            for tt in (range(4) if "scatter" not in _SKIP else []):
                                t = g0 + tt
                                tl = tl0 + tt
                                b = bases[t]
                                segs = []
                                lo = b
                                while lo < b + WIN:
                                    hi = min(b + WIN, (lo // 512 + 1) * 512)
                                    segs.append((lo, hi))
                                    lo = hi
                                for lhs in (
                                    msgt4[:, tt * d : (tt + 1) * d],
                                    gbuf[:, tl, d : 2 * d],
                                ):
                                    for (lo, hi) in segs:
                                        nc.tensor.matmul(
                                            aggT[:, lo:hi],
                                            lhs,
                                            hotv_sb[:, t, lo - b : hi - b],
                                            start=False,
                                            stop=False,
                                        )
                            continue_tiles = []
                        else:
                            continue_tiles = list(range(4))
                        for tt in continue_tiles:
                            t = g0 + tt
                            tl = t4 * 4 + tt  # tile within chunk
                            wp = ps_w.tile([128, D2], f32, tag="w", space="PSUM")
                            if "w" in _SKIP:
                                nc.vector.memset(wp[:, 0:2], 0.0)
                            if "w" not in _SKIP:
                              for wo in range(0, D2, 512):
                                wn = min(512, D2 - wo)
                                nc.tensor.matmul(
                                    wp[:, wo : wo + wn],
                                    hs[:, tt * 128 : (tt + 1) * 128],
                                    ew2pT_sb[L][:, wo : wo + wn],
                                    start=True,
                                    stop=True,
                                )
                            prod = pw_.tile([128, D2], bf16 if PROD_BF16 else f32, tag="prod")
                            msgt = pw_.tile([128, d], f32, tag="msg")
                            if "dve" in _SKIP:
                                nc.vector.memset(prod[:, 0:2], 0.0)
                                nc.vector.memset(msgt[:, 0:2], 0.0)
                            if "dve" not in _SKIP:
                              xp0 = gbuf[:, tl, 0:d]
                              xp_b = bass.AP(
                                tensor=xp0.tensor,
                                offset=xp0.offset,
                                ap=[xp0.ap[0], [0, d], [1, d]],
                              )
                              nc.vector.tensor_tensor(
                                out=prod[:], in0=wp[:], in1=xp_b, op=mybir.AluOpType.mult
                              )
                              nc.vector.tensor_reduce(
                                out=msgt[:],
                                in_=prod[:].rearrange("p (o i) -> p o i", i=d),
                                axis=mybir.AxisListType.X,
                                op=mybir.AluOpType.add,
                              )
                            if "scatter" not in _SKIP:
                              b = bases[t]
                              segs = []
                              lo = b
                              while lo < b + WIN:
                                hi = min(b + WIN, (lo // 512 + 1) * 512)
                                segs.append((lo, hi))
                                lo = hi
                              for lhs in (msgt[:], gbuf[:, tl, d : 2 * d]):
                                for (lo, hi) in segs:
                                    nc.tensor.matmul(
                                        aggT[:, lo:hi],
                                        lhs,
                                        hotv_sb[:, t, lo - b : hi - b],
                                        start=False,
                                        stop=False,
                                    )

                # ---- root + bias into aggT ----
                for (o, n) in NCHUNK:
                    nc.tensor.matmul(
                        aggT[:, o : o + n],
                        wroot_sb[L][:],
                        xpxb_sb[0 : d + 1, o : o + n],
                        start=False,
                        stop=True,
                    )

                # ---- update: x_next = relu(conv) + xproj ----
                relu_sb = pw_.tile([d, NSP], f32, tag="relu")
                nc.scalar.activation(relu_sb[:], aggT[:], mybir.ActivationFunctionType.Relu)
                xT_next = px.tile([d + 1, NSP], f32, tag="xTn")
                nc.vector.memset(xT_next[:], 1.0)
                nc.vector.tensor_tensor(
                    out=xT_next[0:d, :],
                    in0=relu_sb[:],
                    in1=xpxb_sb[0:d, :],
                    op=mybir.AluOpType.add,
                )
                xT_cur = xT_next

            # ---- head: build x4 bf16 gather table ----
            d4 = DIMS[3]
            x4bf = pw_.tile([d4, NSP], bf16, tag="x4bf")
            nc.vector.tensor_copy(out=x4bf[:], in_=xT_cur[0:d4, :])
            for ch in range(10):
                tp = ps_h.tile([128, 512], bf16, tag="h", space="PSUM")
                nc.tensor.transpose(
                    out=tp[:, 0:d4],
                    in_=x4bf[:, ch * 128 : (ch + 1) * 128],
                    identity=id16b_sb[:],
                )
                nc.scalar.activation(
                    x4cat_sb[:, ch, 0:d4], tp[:, 0:d4], mybir.ActivationFunctionType.Copy
                )
            nc.sync.dma_start(
                out=x4in.ap().rearrange("(c p) w -> p c w", p=128), in_=x4cat_sb[:]
            )
            if "ag" not in _SKIP:
                nc.gpsimd.collective_compute(
                    "AllGather",
                    mybir.AluOpType.bypass,
                    replica_groups=[list(range(NCOR))],
                    ins=[x4in[:]],
                    outs=[x4ag[:]],
                )

            # ---- head compute ----
            for ck in (range(4) if "head" not in _SKIP else []):
                gr = pg.tile([128, 1, TC * 128], bf16, tag="g")
                if "gather" in _SKIP:
                    nc.vector.memset(gr[:, :, 0:2], 0.0)
                if "gather" not in _SKIP:
                    nc.gpsimd.dma_gather(
                        gr[:],
                        x4ag[:],
                        ridx_sb[:, ck * TC * 8 : (ck + 1) * TC * 8],
                        TC * 128,
                        TC * 128,
                        GWB,
                        transpose=True,
                        single_packet=False,
                    )
                gc_ = pg.tile([128, 1, TC * 128], bf16, tag="g")
                if "gather" in _SKIP:
                    nc.vector.memset(gc_[:, :, 0:2], 0.0)
                if "gather" not in _SKIP:
                    nc.gpsimd.dma_gather(
                        gc_[:],
                        x4ag[:],
                        cidx_sb[:, ck * TC * 8 : (ck + 1) * TC * 8],
                        TC * 128,
                        TC * 128,
                        GWB,
                        transpose=True,
                        single_packet=False,
                    )
                for s in range(TC * 128 // 512):
                    sl = slice(s * 512, (s + 1) * 512)
                    hh = ps_w.tile([16, 512], f32, tag="w", space="PSUM")
                    nc.tensor.matmul(hh[:], wh0a_sb[:], gr[0:17, 0, sl], start=True, stop=False)
                    nc.tensor.matmul(hh[:], wh0b_sb[:], gc_[0:17, 0, sl], start=False, stop=True)
                    hhs = pw_.tile([17, 512], f32, tag="hh")
                    nc.vector.memset(hhs[:], 1.0)
                    nc.scalar.activation(hhs[0:16, :], hh[:], mybir.ActivationFunctionType.Relu)
                    op = ps_w.tile([1, 512], f32, tag="w", space="PSUM")
                    nc.tensor.matmul(op[:], wh1_sb[:], hhs[:], start=True, stop=True)
                    oc = pw_.tile([1, 512], f32, tag="oc")
                    nc.scalar.activation(oc[:], op[:], mybir.ActivationFunctionType.Copy)
                    off = ck * TC * 128 + s * 512
                    nc.sync.dma_start(out=out_d[:, off : off + 512], in_=oc[:])

    nc.compile()
    return nc


_CACHE = {}


def kernel(**inputs):
    x = np.asarray(inputs["x"])
    edge_attr = np.asarray(inputs["edge_attr"])
    edge_index = np.asarray(inputs["edge_index"])
    params = []
    for i in range(4):
        params.append(tuple(
            np.asarray(inputs[k]) for k in (
                f"p_w{i}", f"p_b{i}", f"e1_w{i}", f"e1_b{i}",
                f"e2_w{i}", f"e2_b{i}", f"r_w{i}", f"c_b{i}",
            )
        ))
    m_w0 = np.asarray(inputs["m_w0"]); m_b0 = np.asarray(inputs["m_b0"])
    m_w1 = np.asarray(inputs["m_w1"]); m_b1 = np.asarray(inputs["m_b1"])

    T, bases, in_maps, keep = _host_prep(
        x, edge_attr, edge_index, params, m_w0, m_b0, m_w1, m_b1
    )

    key = (T, hashlib.sha1(edge_index.tobytes()).hexdigest())
    if key not in _CACHE:
        _CACHE[key] = _build_program(T, bases)
    nc = _CACHE[key]

    res = run_bass_kernel_spmd(nc, in_maps, core_ids=list(range(NCOR)))
    out = np.zeros(E, dtype=np.float32)
    for c in range(NCOR):
        oc = np.asarray(res.results[c]["out"]).reshape(-1)
        mask = keep[c] >= 0
        out[keep[c][mask]] = oc[mask]
    return out

